# revision 18
# baseline (speedup 1.0000x reference)
"""CAWformer forward on 8 TRN2 NeuronCores — data parallel over batch.

Math notes (all exact algebraic rewrites of the reference):
  * irfft(xf_i * conj(xf_j)).mean(-1) == s_i * s_j / DM with s = x.sum(-1),
    so the FFT cross-correlation attention is softmax(outer(s, s)/c) @ x.
  * The 8-shift auto-attention: scores_i = <q@Wk, roll_i(x)> (+const that
    cancels in softmax); out = (sum_i p_i roll_i(x)) @ Wv.T @ Wo.T + const.
  * The depthwise smoothing conv is a (T,T) band matrix S; residual embed
    folds to inp[b].T @ (R.T @ emb_W.T) with R = I - S.

Perf structure (v1 rewrite):
  * weights live in DRAM as bf16 (half the HBM traffic; PE runs bf16 and
    fp32r at the same 1 col/cycle for wide outputs), double-buffered.
  * residual adds and bias adds are folded into the PSUM accumulation as
    ident@x / ones-row outer-product matmuls (PE has slack, DVE does not).
  * 128x128 PE transposes land in one (128,512) PSUM tile per group and
    leave via ONE wide copy; copies/elementwise split DVE (c=0) vs
    Pool/GpSimd (c=1) so the two batch chains use different engines.
  * x is materialized twice side by side (x_dup) so the 8 circular-shift
    score dots and p-weighted-roll matmuls need no wraparound splits.
  * ACT tables: Exp/Gelu switched 4x per layer, prewarmed off the critical
    path; LN rstd leaves the ACT Sqrt table (env KRSTD: pow|sqrt).
"""

import os
import numpy as np

B, T, C, DM, L, P, KS = 16, 512, 128, 512, 3, 64, 25
EPS = 1e-5
NS = DM // P           # 8 circular shifts
NC_ = 8                # cores
BPC = B // NC_         # batches per core = 2
H = 2 * DM             # FFN hidden = 1024
KD = DM // 128         # 4 k-tiles over d_model
KH = H // 128          # 8 k-tiles over hidden

RSTD_MODE = os.environ.get("KRSTD", "sqrt")


def _build(nc, tile, mybir, bass):
    F32 = mybir.dt.float32
    R32 = mybir.dt.float32r
    B16 = mybir.dt.bfloat16
    AT = mybir.ActivationFunctionType
    ALU = mybir.AluOpType
    AX = mybir.AxisListType

    def mmr(out, lhsT, rhs, start, stop):
        nc.tensor.matmul(out, lhsT, rhs, start=start, stop=stop)

    # ---------------- DRAM I/O ----------------
    d = {}
    def din(name, shape, dt_=None):
        d[name] = nc.dram_tensor(name, list(shape), dt_ or F32, kind="ExternalInput")
        return d[name]

    din("xin", (128, BPC, KD, C), B16)
    din("memb", (128, KD, DM), B16)
    din("wpos", (C, DM))
    din("ident", (128, 128), R32)
    din("identb", (128, 128), B16)
    din("vw1t", (L, 128, KD, H), B16); din("vw2t", (L, 128, KH, DM), B16)
    din("aw1t", (L, 128, KD, H), B16); din("aw2t", (L, 128, KH, DM), B16)
    din("m1", (L, 128, KD, DM), B16); din("m2", (L, 128, KD, DM), B16)
    din("vb1", (L, 128, KH)); din("ab1", (L, 128, KH))
    din("vb2", (L, DM)); din("ab2", (L, DM))
    din("c1", (L, DM)); din("c2", (L, DM))
    din("vsb", (L, 1)); din("asb", (L, 1))
    din("vgc", (L, C)); din("vbc", (L, C)); din("vgl", (L, DM)); din("vbl", (L, DM))
    din("agc", (L, C)); din("abc", (L, C)); din("agl", (L, DM)); din("abl", (L, DM))
    out_d = nc.dram_tensor("out", [BPC, C, DM], F32, kind="ExternalOutput")

    def bc_ap(src, parts=128):
        return bass.AP(tensor=src.tensor, offset=src.offset,
                       ap=[[0, parts]] + [list(x) for x in src.ap])

    def col_ap(src):
        return bass.AP(tensor=src.tensor, offset=src.offset,
                       ap=[list(src.ap[0]), [0, 1]])

    with tile.TileContext(nc) as tc:
        import contextlib
        ctx = contextlib.ExitStack()
        with ctx:
            wp = ctx.enter_context(tc.tile_pool(name="wp", bufs=2))
            ap_ = ctx.enter_context(tc.tile_pool(name="ap", bufs=1))
            bcp = ctx.enter_context(tc.tile_pool(name="bcp", bufs=8))
            sp = ctx.enter_context(tc.tile_pool(name="sp", bufs=8))
            cp = ctx.enter_context(tc.tile_pool(name="cp", bufs=1))
            pbig = ctx.enter_context(tc.tile_pool(name="pbig", bufs=4, space="PSUM"))
            ph = ctx.enter_context(tc.tile_pool(name="ph", bufs=2, space="PSUM"))
            pt = ctx.enter_context(tc.tile_pool(name="pt", bufs=2, space="PSUM"))

            # ---------------- constants ----------------
            xin_sb = cp.tile([128, BPC, KD, C], B16)
            for c in range(BPC):
                nc.sync.dma_start(out=xin_sb[:, c], in_=d["xin"].ap()[:, c])
            memb_sb = cp.tile([128, KD, DM], B16)
            nc.sync.dma_start(out=memb_sb, in_=d["memb"].ap())
            wpos_sb = cp.tile([128, DM], F32)
            nc.sync.dma_start(out=wpos_sb, in_=d["wpos"].ap())
            ident = cp.tile([128, 128], R32)
            nc.sync.dma_start(out=ident, in_=d["ident"].ap())
            identb = cp.tile([128, 128], B16)
            nc.sync.dma_start(out=identb, in_=d["identb"].ap())
            epsc = cp.tile([128, 1], F32)
            nc.vector.memset(epsc, EPS)
            magic = cp.tile([128, 1], mybir.dt.int32)
            nc.vector.memset(magic, 0x5f3759df)
            c15 = cp.tile([128, 1], F32)
            nc.vector.memset(c15, 1.5)

            # ---------------- embed:  x[c] = xin[c].T @ memb + wpos ----------------
            x_t = ap_.tile([128, BPC, DM], R32, tag="x", bufs=2)
            for c in range(BPC):
                x_ps = pbig.tile([128, DM], F32, tag="big")
                for k in range(KD):
                    mmr(x_ps, xin_sb[:, c, k, :], memb_sb[:, k, :],
                        start=(k == 0), stop=(k == KD - 1))
                nc.vector.tensor_add(x_t[:, c, :], x_ps, wpos_sb)

            inv_sqc = float(1.0 / np.sqrt(DM * np.sqrt(DM)))
            phase = os.environ.get("KPHASE", "full")
            srow_of = {}

            def ve(c):
                return nc.vector if c == 0 else nc.gpsimd

            def psum_copy(c, out, in_):
                # Pool cannot access PSUM: c0 -> DVE, c1 -> ACT Identity
                if c == 0:
                    nc.vector.tensor_copy(out, in_)
                else:
                    nc.scalar.activation(out, in_, AT.Identity)

            # ---------------- layers ----------------
            def load_weights(l):
                w = {}
                w["vw1t"] = wp.tile([128, KD, H], B16, tag="vw1t", name=f"vw1t{l}")
                nc.sync.dma_start(out=w["vw1t"], in_=d["vw1t"][l])
                w["vw2t"] = wp.tile([128, KH, DM], B16, tag="vw2t", name=f"vw2t{l}")
                nc.sync.dma_start(out=w["vw2t"], in_=d["vw2t"][l])
                w["m1"] = wp.tile([128, KD, DM], B16, tag="m1", name=f"m1_{l}")
                nc.sync.dma_start(out=w["m1"], in_=d["m1"][l])
                w["m2"] = wp.tile([128, KD, DM], B16, tag="m2", name=f"m2_{l}")
                nc.sync.dma_start(out=w["m2"], in_=d["m2"][l])
                w["aw1t"] = wp.tile([128, KD, H], B16, tag="aw1t", name=f"aw1t{l}")
                nc.sync.dma_start(out=w["aw1t"], in_=d["aw1t"][l])
                w["aw2t"] = wp.tile([128, KH, DM], B16, tag="aw2t", name=f"aw2t{l}")
                nc.sync.dma_start(out=w["aw2t"], in_=d["aw2t"][l])
                w["vb1"] = sp.tile([128, KH], F32, tag="vb1", bufs=2, name=f"vb1_{l}")
                nc.sync.dma_start(out=w["vb1"], in_=d["vb1"][l])
                w["ab1"] = sp.tile([128, KH], F32, tag="ab1", bufs=2, name=f"ab1_{l}")
                nc.sync.dma_start(out=w["ab1"], in_=d["ab1"][l])
                return w

            wts = load_weights(0) if phase != "emb" else None
            for l in range(L if phase == "full" else 1):
                if phase == "emb":
                    break
                vw1t, vw2t, m1, m2, aw1t, aw2t, vb1, ab1 = (
                    wts["vw1t"], wts["vw2t"], wts["m1"], wts["m2"],
                    wts["aw1t"], wts["aw2t"], wts["vb1"], wts["ab1"])

                vgc = sp.tile([128, 1], F32, tag="vgc")
                nc.gpsimd.dma_start(out=vgc, in_=col_ap(d["vgc"][l]))
                agc = sp.tile([128, 1], F32, tag="agc")
                nc.gpsimd.dma_start(out=agc, in_=col_ap(d["agc"][l]))
                vbc = sp.tile([128, 1], F32, tag="vbc")
                nc.gpsimd.dma_start(out=vbc, in_=col_ap(d["vbc"][l]))
                abc = sp.tile([128, 1], F32, tag="abc")
                nc.gpsimd.dma_start(out=abc, in_=col_ap(d["abc"][l]))

                def bcast(name):
                    t = bcp.tile([128, DM], F32, tag="bc", name=f"{name}_bc{l}")
                    eng = nc.gpsimd if l == 0 else nc.sync
                    eng.dma_start(out=t, in_=bc_ap(d[name][l]))
                    return t
                vglb = bcast("vgl"); vblb = bcast("vbl")
                aglb = bcast("agl"); ablb = bcast("abl")
                vb2b = bcast("vb2"); ab2b = bcast("ab2")
                c1b = bcast("c1"); c2b = bcast("c2")

                gcI = sp.tile([128, 128], R32, tag="gcI", bufs=2)
                nc.vector.tensor_scalar_mul(gcI, ident, vgc)
                vsb = sp.tile([128, 1], F32, tag="vsb")
                nc.gpsimd.dma_start(out=vsb, in_=bc_ap(d["vsb"][l]))
                asb = sp.tile([128, 1], F32, tag="asb")
                nc.gpsimd.dma_start(out=asb, in_=bc_ap(d["asb"][l]))

                # ============ VarCor block ============
                cT = ap_.tile([128, BPC, 128], R32, tag="cT")
                for c in range(BPC):
                    if c in srow_of:
                        srow = srow_of[c]
                    else:
                        srow = sp.tile([128, 1], F32, tag="srow", bufs=4)
                        nc.vector.tensor_reduce(srow, x_t[:, c, :], AX.X, ALU.add)
                    s2 = sp.tile([128, 1], R32, tag="s2", bufs=4)
                    nc.scalar.mul(s2, srow, inv_sqc)
                    sT_ps = pbig.tile([1, 128], R32, tag="big", name=f"sTps{l}_{c}")
                    nc.tensor.transpose(sT_ps, s2, ident)
                    sT = sp.tile([1, 128], R32, tag="sT", bufs=4)
                    nc.scalar.activation(sT, sT_ps, AT.Identity)
                    corr_ps = pbig.tile([128, 128], F32, tag="big", name=f"corrps{l}_{c}")
                    mmr(corr_ps, sT, sT, start=True, stop=True)
                    corrE = ap_.tile([128, 128], F32, tag="corrE", bufs=2)
                    rsum = sp.tile([128, 1], F32, tag="rsum", bufs=4)
                    nc.scalar.activation(corrE, corr_ps, AT.Exp, accum_out=rsum)
                    rinv = sp.tile([128, 1], F32, tag="rinv", bufs=4)
                    nc.vector.reciprocal(rinv, rsum)
                    corrBN = ap_.tile([128, 128], R32, tag="corrBN", bufs=2)
                    nc.vector.tensor_scalar(corrBN, corrE, rinv, vgc, ALU.mult, ALU.mult)
                    # cT = corrBN^T + gcI^T  (gcI diagonal: transpose == itself)
                    cT_ps = pbig.tile([128, 128], R32, tag="big", name=f"cTps{l}_{c}")
                    nc.tensor.matmul(cT_ps, corrBN, ident, is_transpose=True,
                                     start=True, stop=False)
                    nc.tensor.matmul(cT_ps, gcI, ident, is_transpose=True,
                                     start=False, stop=True)
                    psum_copy(c, cT[:, c, :], cT_ps)

                # prewarm Gelu table; input aliases c1's corrE so the
                # scheduler cannot hoist this above the last Exp use
                pw_g = sp.tile([128, 1], F32, tag="pw", bufs=4, name=f"pwg{l}")
                nc.scalar.activation(pw_g, corrE[:, 0:1], AT.Gelu)

                # r2 rows-major via corr matmul; feature-major via transposes
                r2r = ap_.tile([128, BPC, DM], B16, tag="r2r")
                r2T = ap_.tile([128, KD, 2 * 128], B16, tag="r2T")
                for c in range(BPC):
                    rr_ps = pbig.tile([128, DM], F32, tag="big", name=f"rrps{l}_{c}")
                    mmr(rr_ps, cT[:, c, :], x_t[:, c, :], start=True, stop=True)
                    nc.scalar.activation(r2r[:, c, :], rr_ps, AT.Identity, bias=vbc)
                    rt_ps = pt.tile([128, KD, 128], B16, tag="t", name=f"rtps{l}_{c}")
                    for m in range(KD):
                        nc.tensor.matmul(rt_ps[:, m, :],
                                         r2r[:, c, m * 128:(m + 1) * 128], identb,
                                         is_transpose=True, start=True, stop=True)
                    psum_copy(c, r2T[:, :, c * 128:(c + 1) * 128], rt_ps)

                if phase == "corr":
                    x_t = r2r
                    break
                x_t = _ffn_ln(nc, tile, mybir, bass, tc, ap_, sp, bcp, ph, pbig, pt,
                              r2T, r2r, vw1t, vb1, vw2t, vb2b, vglb, vblb, l, "v",
                              epsc, vsb, srow_of, ident, magic, c15, ve,
                              dup=True)
                if phase == "vc0":
                    break
                if l + 1 < L and phase == "full":
                    wts = load_weights(l + 1)

                # ============ Auto-attention block ============
                # x_t here is x_dup [128, BPC, 2*DM]; x view is first half
                x_dup = x_t
                def xv(c):
                    return x_dup[:, c, 0:DM]

                # xT feature-major via batched transposes
                xT = ap_.tile([128, KD, 2 * 128], B16, tag="xT")
                for c in range(BPC):
                    xt_ps = pt.tile([128, KD, 128], B16, tag="t", name=f"xtps{l}_{c}")
                    for m in range(KD):
                        nc.tensor.matmul(xt_ps[:, m, :],
                                         xv(c)[:, m * 128:(m + 1) * 128], identb,
                                         is_transpose=True, start=True, stop=True)
                    psum_copy(c, xT[:, :, c * 128:(c + 1) * 128], xt_ps)

                # u = x @ M1 + c1   (rows-major out; c1 via ones-fold)
                u_t = ap_.tile([128, BPC, DM], F32, tag="u")
                for c in range(BPC):
                    u_ps = pbig.tile([128, DM], F32, tag="big", name=f"ups{l}_{c}")
                    for k in range(KD):
                        mmr(u_ps, xT[:, k, c * 128:(c + 1) * 128], m1[:, k, :],
                            start=(k == 0), stop=(k == KD - 1))
                    nc.vector.tensor_add(u_t[:, c, :], u_ps, c1b)

                if phase == "u":
                    x_t = u_t
                    break

                # scores S[r,i] = <u, roll_i(x)> * DM^-0.5 ; softmax over i
                scl = float(DM ** -0.5)
                Se_of = {}
                sinv_of = {}
                for c in range(BPC):
                    trash = ap_.tile([128, DM], F32, tag="trash", bufs=1,
                                     name=f"tr{l}_{c}")
                    Sa = sp.tile([128, NS], F32, tag="Sa", bufs=2)
                    for i in range(NS):
                        nc.vector.scalar_tensor_tensor(
                            out=trash, in0=u_t[:, c, :], scalar=scl,
                            in1=x_dup[:, c, P * i:P * i + DM],
                            op0=ALU.mult, op1=ALU.mult,
                            accum_out=Sa[:, i:i + 1])
                    Se = sp.tile([128, NS], F32, tag="Se", bufs=2,
                                 name=f"Se{l}_{c}")
                    ssum = sp.tile([128, 1], F32, tag="ssum", bufs=4)
                    nc.scalar.activation(Se, Sa, AT.Exp, accum_out=ssum)
                    sinv = sp.tile([128, 1], F32, tag="sinv", bufs=4,
                                   name=f"sinv{l}_{c}")
                    nc.vector.reciprocal(sinv, ssum)
                    Se_of[c] = Se
                    sinv_of[c] = sinv

                # prewarm Gelu for the a-FFN; dep-anchored on c1's Se
                pw_g2 = sp.tile([128, 1], F32, tag="pw", bufs=4, name=f"pwg2{l}")
                nc.scalar.activation(pw_g2, Se_of[BPC - 1][:, 0:1], AT.Gelu)

                if phase == "sc":
                    x_t = ap_.tile([128, BPC, DM], F32, tag="scdump")
                    nc.vector.memset(x_t, 0.0)
                    for c in range(BPC):
                        nc.vector.tensor_scalar(x_t[:, c, 0:NS], Se_of[c],
                                                sinv_of[c], None, ALU.mult)
                    break

                # vm = sum_i p_i roll_i(x): diag(p_i) bf16, x_dup slices wide
                vm_t = ap_.tile([128, BPC, DM], B16, tag="vm")
                for c in range(BPC):
                    dgs = ap_.tile([128, NS, 128], B16, tag="dg", bufs=2,
                                   name=f"dgs{l}_{c}")
                    for i in range(NS):
                        if c == 0:
                            nc.vector.tensor_scalar_mul(dgs[:, i, :], identb,
                                                        Se_of[c][:, i:i + 1])
                        else:
                            nc.scalar.activation(dgs[:, i, :], identb, AT.Identity,
                                                 scale=Se_of[c][:, i:i + 1])
                    vm_ps = pbig.tile([128, DM], F32, tag="big", name=f"vmps{l}_{c}")
                    for i in range(NS):
                        mmr(vm_ps, dgs[:, i, :], x_dup[:, c, P * i:P * i + DM],
                            start=(i == 0), stop=(i == NS - 1))
                    # normalize by 1/sum(exp) fused into the PSUM evacuation
                    nc.vector.tensor_scalar(vm_t[:, c, :], vm_ps, sinv_of[c],
                                            None, ALU.mult)

                if phase == "vm":
                    x_t = vm_t
                    break

                # vmT feature-major
                vmT = ap_.tile([128, KD, 2 * 128], B16, tag="vmT")
                for c in range(BPC):
                    vt_ps = pt.tile([128, KD, 128], B16, tag="t", name=f"vtps{l}_{c}")
                    for m in range(KD):
                        nc.tensor.matmul(vt_ps[:, m, :],
                                         vm_t[:, c, m * 128:(m + 1) * 128], identb,
                                         is_transpose=True, start=True, stop=True)
                    psum_copy(c, vmT[:, :, c * 128:(c + 1) * 128], vt_ps)

                # o = vm @ M2 + c2 + x (residual via ident-fold); r1 = BN(o)
                # xc2 = x + c2 precomputed off the critical path (Pool)
                xc2 = ap_.tile([128, BPC, DM], F32, tag="xc2")
                for c in range(BPC):
                    nc.gpsimd.tensor_add(xc2[:, c, :], xv(c), c2b)
                r1r = ap_.tile([128, BPC, DM], B16, tag="r1r")
                for c in range(BPC):
                    o_ps = pbig.tile([128, DM], F32, tag="big", name=f"ops{l}_{c}")
                    for k in range(KD):
                        mmr(o_ps, vmT[:, k, c * 128:(c + 1) * 128], m2[:, k, :],
                            start=(k == 0), stop=(k == KD - 1))
                    t1 = ap_.tile([128, DM], F32, tag="t1", bufs=2, name=f"t1{l}_{c}")
                    nc.vector.tensor_add(t1, o_ps, xc2[:, c, :])
                    nc.scalar.activation(r1r[:, c, :], t1, AT.Identity,
                                         bias=abc, scale=agc)

                if phase == "attn":
                    x_t = r1r
                    break

                # r1T feature-major
                r1T = ap_.tile([128, KD, 2 * 128], B16, tag="r1T")
                for c in range(BPC):
                    r1_ps = pt.tile([128, KD, 128], B16, tag="t", name=f"r1ps{l}_{c}")
                    for m in range(KD):
                        nc.tensor.matmul(r1_ps[:, m, :],
                                         r1r[:, c, m * 128:(m + 1) * 128], identb,
                                         is_transpose=True, start=True, stop=True)
                    psum_copy(c, r1T[:, :, c * 128:(c + 1) * 128], r1_ps)

                x_t = _ffn_ln(nc, tile, mybir, bass, tc, ap_, sp, bcp, ph, pbig, pt,
                              r1T, r1r, aw1t, ab1, aw2t, ab2b, aglb, ablb, l, "a",
                              epsc, asb, srow_of, ident, magic, c15, ve,
                              dup=False)

            # ---------------- store ----------------
            for c in range(BPC):
                nc.sync.dma_start(out=out_d.ap()[c],
                                  in_=x_t[:, c, 0:DM].bitcast(F32))


def _ffn_ln(nc, tile, mybir, bass, tc, ap_, sp, bcp, ph, pbig, pt,
            rT, rrows, w1t, b1c, w2t, b2b, glb, blb, l, pfx, epsc,
            sumb, srow_of, ident, magic, c15, ve, dup):
    """h = gelu(r @ W1.T + b1); y = h @ W2.T + b2; x = LN(y + r) * g + b.

    b1/b2/residual folded into the PSUM accumulation. LN stats via bn_stats
    on the PSUM z; rstd without the ACT Sqrt table (KRSTD=pow) or with it
    (KRSTD=sqrt). If dup, output x is [128, BPC, 2*DM] (two copies side by
    side) for the shift-window consumers."""
    F32 = mybir.dt.float32
    R32 = mybir.dt.float32r
    B16 = mybir.dt.bfloat16
    AT = mybir.ActivationFunctionType
    ALU = mybir.AluOpType

    # FFN1: hT hidden-major; one LDWEIGHTS per (k, mh) chunk covers both c
    hT = ap_.tile([128, KH, 2 * 128], B16, tag="hT", bufs=2, name=f"hT{pfx}{l}")
    for mh2 in range(KH // 2):
        h_ps = ph.tile([128, 2, 128 * 2], F32, tag="h", name=f"hps{pfx}{l}_{mh2}")
        for half in range(2):
            mh = mh2 * 2 + half
            for k in range(KD):
                nc.tensor.matmul(h_ps[:, half, :], w1t[:, k, mh * 128:(mh + 1) * 128],
                                 rT[:, k, :], start=(k == 0), stop=(k == KD - 1))
            nc.scalar.activation(hT[:, mh, :], h_ps[:, half, :], AT.Gelu,
                                 bias=b1c[:, mh:mh + 1])

    xw = 2 * DM if dup else DM
    x_new = ap_.tile([128, BPC, xw], B16 if dup else R32,
                     tag="xd" if dup else "x", bufs=2, name=f"x{pfx}{l}")
    # rb = r + b2 precomputed off the critical path (Pool)
    rb = ap_.tile([128, BPC, DM], F32, tag="rb", bufs=1, name=f"rb{pfx}{l}")
    for c in range(BPC):
        nc.gpsimd.tensor_add(rb[:, c, :], rrows[:, c, :], b2b)
    for c in range(BPC):
        y_ps = pbig.tile([128, DM], F32, tag="big", name=f"yps{pfx}{l}_{c}")
        for k in range(KH):
            nc.tensor.matmul(y_ps, hT[:, k, c * 128:(c + 1) * 128],
                             w2t[:, k, :], start=(k == 0), stop=(k == KH - 1))
        z = ap_.tile([128, DM], F32, tag="z", bufs=2, name=f"z{pfx}{l}_{c}")
        nc.vector.tensor_add(z, y_ps, rb[:, c, :])
        st6 = sp.tile([128, 6], F32, tag="st6", bufs=4)
        nc.vector.bn_stats(out=st6, in_=z)
        mv = sp.tile([128, 2], F32, tag="mv", bufs=4)
        nc.vector.bn_aggr(out=mv, in_=st6)
        vef = sp.tile([128, 1], F32, tag="vef", bufs=4)
        nc.vector.tensor_scalar(vef, mv[:, 1:2], EPS, None, ALU.add)
        veh = sp.tile([128, 1], F32, tag="veh", bufs=4)
        nc.vector.tensor_scalar(veh, vef, 0.5, None, ALU.mult)
        yi = sp.tile([128, 1], mybir.dt.int32, tag="yi", bufs=4)
        nc.vector.tensor_scalar(yi, vef.bitcast(mybir.dt.int32), 1, None,
                                ALU.logical_shift_right)
        y0 = sp.tile([128, 1], mybir.dt.int32, tag="y0", bufs=4)
        nc.vector.tensor_sub(y0, magic, yi)
        y0f = y0.bitcast(F32)
        q0 = sp.tile([128, 1], F32, tag="q0", bufs=4)
        nc.vector.tensor_mul(q0, y0f, y0f)
        w0 = sp.tile([128, 1], F32, tag="w0", bufs=4)
        nc.vector.scalar_tensor_tensor(out=w0, in0=q0, scalar=veh, in1=c15,
                                       op0=ALU.mult, op1=ALU.subtract)
        y1 = sp.tile([128, 1], F32, tag="y1", bufs=4)
        nc.vector.tensor_mul(y1, y0f, w0)
        q1 = sp.tile([128, 1], F32, tag="q1", bufs=4)
        nc.vector.tensor_mul(q1, y1, y1)
        w1 = sp.tile([128, 1], F32, tag="w1", bufs=4)
        nc.vector.scalar_tensor_tensor(out=w1, in0=q1, scalar=veh, in1=c15,
                                       op0=ALU.mult, op1=ALU.subtract)
        rstd = sp.tile([128, 1], F32, tag="rstd", bufs=4)
        nc.vector.tensor_mul(rstd, y1, w1)
        xn = ap_.tile([128, DM], F32, tag="xn", bufs=2, name=f"xn{pfx}{l}_{c}")
        nc.vector.tensor_scalar(xn, z, mv[:, 0:1], rstd, ALU.subtract, ALU.mult)
        if pfx == "a" and l < L - 1:
            trash2 = ap_.tile([128, DM], F32, tag="trash2", bufs=1,
                              name=f"tr2{pfx}{l}_{c}")
            sraw = sp.tile([128, 1], F32, tag="sraw", bufs=4)
            nc.vector.scalar_tensor_tensor(
                out=trash2, in0=xn, scalar=1.0, in1=glb,
                op0=ALU.mult, op1=ALU.mult, accum_out=sraw)
            srow = sp.tile([128, 1], F32, tag="srow", bufs=4, name=f"srow{pfx}{l}_{c}")
            nc.scalar.activation(srow, sraw, AT.Identity, bias=sumb)
            srow_of[c] = srow
        # affine x = xn*g + b  (c0 on DVE, c1 on Pool)
        ve(c).tensor_mul(x_new[:, c, 0:DM], xn, glb)
        ve(c).tensor_add(x_new[:, c, 0:DM], x_new[:, c, 0:DM], blb)
        if dup:
            nc.vector.tensor_copy(x_new[:, c, DM:2 * DM], x_new[:, c, 0:DM])
    # prewarm Exp for the next softmax; dep-anchored on the last gelu tile.
    # Skipped after the final FFN (nothing needs Exp again).
    if not (pfx == "a" and l == L - 1):
        pw_e = sp.tile([128, 1], F32, tag="pw", bufs=4, name=f"pwe{pfx}{l}")
        nc.scalar.activation(pw_e, hT[:, KH - 1, 255:256], AT.Exp)
    return x_new


# ======================================================================
# host side
# ======================================================================

_COMPILED = {}


def _compile():
    if "nc" in _COMPILED:
        return _COMPILED["nc"]
    import concourse.bass as bass
    import concourse.bacc as bacc
    import concourse.tile as tile
    from concourse import mybir
    nc = bacc.Bacc("TRN2", target_bir_lowering=False, debug=False, num_devices=NC_)
    _build(nc, tile, mybir, bass)
    nc.compile()
    _COMPILED["nc"] = nc
    return nc


def _host_prep(inputs):
    import ml_dtypes
    bf16 = ml_dtypes.bfloat16
    f = lambda k: np.asarray(inputs[k], np.float32)
    ld_w = f("ld_w").reshape(KS).astype(np.float64)
    S = np.zeros((T, T), np.float64)
    idx = np.clip(np.arange(T)[:, None] + np.arange(KS)[None, :] - KS // 2, 0, T - 1)
    for k in range(KS):
        np.add.at(S, (np.arange(T), idx[:, k]), ld_w[k])
    Rm = np.eye(T) - S
    emb_W = f("emb_W").astype(np.float64)
    memb = (Rm.T @ emb_W.T).astype(np.float32)              # (T, DM)
    wpos = (f("W_pos") + f("emb_b")[None, :]
            - float(f("ld_b")[0]) * emb_W.sum(1).astype(np.float32)[None, :])

    def shuf(a):
        kn, n = a.shape
        return a.reshape(kn // 128, 128, n).transpose(1, 0, 2)

    g = {"memb": np.ascontiguousarray(shuf(memb)).astype(bf16),
         "wpos": np.ascontiguousarray(wpos.astype(np.float32)),
         "ident": np.eye(128, dtype=np.float32),
         "identb": np.eye(128).astype(bf16)}

    s1 = np.float32(1.0 / np.sqrt(1.0 + EPS))
    def stack(fn, dt=np.float32):
        return np.ascontiguousarray(np.stack([fn(l) for l in range(L)]).astype(dt))

    g["vw1t"] = stack(lambda l: shuf(f("vc_W1")[l].T), bf16)
    g["vw2t"] = stack(lambda l: shuf(f("vc_W2")[l].T), bf16)
    g["aw1t"] = stack(lambda l: shuf(f("aa_W1")[l].T), bf16)
    g["aw2t"] = stack(lambda l: shuf(f("aa_W2")[l].T), bf16)
    g["m1"] = stack(lambda l: shuf(f("aa_Wq")[l].astype(np.float64).T
                                   @ f("aa_Wk")[l].astype(np.float64)), bf16)
    g["m2"] = stack(lambda l: shuf((f("aa_Wo")[l].astype(np.float64)
                                    @ f("aa_Wv")[l].astype(np.float64)).T), bf16)
    g["vb1"] = stack(lambda l: f("vc_b1")[l].reshape(KH, 128).T)
    g["ab1"] = stack(lambda l: f("aa_b1")[l].reshape(KH, 128).T)
    g["vb2"] = stack(lambda l: f("vc_b2")[l])
    g["ab2"] = stack(lambda l: f("aa_b2")[l])
    g["c1"] = stack(lambda l: f("aa_bq")[l].astype(np.float64)
                              @ f("aa_Wk")[l].astype(np.float64))
    g["c2"] = stack(lambda l: f("aa_bv")[l].astype(np.float64)
                              @ f("aa_Wo")[l].astype(np.float64).T
                              + f("aa_bo")[l].astype(np.float64))
    g["vsb"] = stack(lambda l: f("vc_ln_b")[l].sum(keepdims=True))
    g["asb"] = stack(lambda l: f("aa_ln_b")[l].sum(keepdims=True))
    g["vgc"] = stack(lambda l: f("vc_bn_g")[l] * s1)
    g["vbc"] = stack(lambda l: f("vc_bn_b")[l])
    g["vgl"] = stack(lambda l: f("vc_ln_g")[l])
    g["vbl"] = stack(lambda l: f("vc_ln_b")[l])
    g["agc"] = stack(lambda l: f("aa_bn_g")[l] * s1)
    g["abc"] = stack(lambda l: f("aa_bn_b")[l])
    g["agl"] = stack(lambda l: f("aa_ln_g")[l])
    g["abl"] = stack(lambda l: f("aa_ln_b")[l])
    return g


def kernel(**inputs):
    from concourse.bass_utils import run_bass_kernel_spmd
    nc = _compile()
    g = _host_prep(inputs)
    inp = np.asarray(inputs["inp"], np.float32)
    in_maps = []
    for core in range(NC_):
        m = dict(g)
        sl = inp[core * BPC:(core + 1) * BPC]          # (BPC, T, C)
        m["xin"] = np.ascontiguousarray(
            sl.reshape(BPC, KD, 128, C).transpose(2, 0, 1, 3)).astype(g["memb"].dtype)
        in_maps.append(m)
    res = run_bass_kernel_spmd(nc, in_maps, core_ids=list(range(NC_)))
    if res.exec_time_ns is not None:
        kernel.last_exec_time_ns = res.exec_time_ns
    out = np.concatenate([res.results[k]["out"] for k in range(NC_)], axis=0)
    return out


kernel.last_exec_time_ns = None


# revision 19
# speedup vs baseline: 1.0466x; 1.0466x over previous
"""CAWformer forward on 8 TRN2 NeuronCores — data parallel over batch.

Math notes (all exact algebraic rewrites of the reference):
  * irfft(xf_i * conj(xf_j)).mean(-1) == s_i * s_j / DM with s = x.sum(-1),
    so the FFT cross-correlation attention is softmax(outer(s, s)/c) @ x.
  * The 8-shift auto-attention: scores_i = <q@Wk, roll_i(x)> (+const that
    cancels in softmax); out = (sum_i p_i roll_i(x)) @ Wv.T @ Wo.T + const.
  * The depthwise smoothing conv is a (T,T) band matrix S; residual embed
    folds to inp[b].T @ (R.T @ emb_W.T) with R = I - S.

Perf structure (v1 rewrite):
  * weights live in DRAM as bf16 (half the HBM traffic; PE runs bf16 and
    fp32r at the same 1 col/cycle for wide outputs), double-buffered.
  * residual adds and bias adds are folded into the PSUM accumulation as
    ident@x / ones-row outer-product matmuls (PE has slack, DVE does not).
  * 128x128 PE transposes land in one (128,512) PSUM tile per group and
    leave via ONE wide copy; copies/elementwise split DVE (c=0) vs
    Pool/GpSimd (c=1) so the two batch chains use different engines.
  * x is materialized twice side by side (x_dup) so the 8 circular-shift
    score dots and p-weighted-roll matmuls need no wraparound splits.
  * ACT tables: Exp/Gelu switched 4x per layer, prewarmed off the critical
    path; LN rstd leaves the ACT Sqrt table (env KRSTD: pow|sqrt).
"""

import os
import numpy as np

B, T, C, DM, L, P, KS = 16, 512, 128, 512, 3, 64, 25
EPS = 1e-5
NS = DM // P           # 8 circular shifts
NC_ = 8                # cores
BPC = B // NC_         # batches per core = 2
H = 2 * DM             # FFN hidden = 1024
KD = DM // 128         # 4 k-tiles over d_model
KH = H // 128          # 8 k-tiles over hidden

RSTD_MODE = os.environ.get("KRSTD", "sqrt")


def _build(nc, tile, mybir, bass):
    F32 = mybir.dt.float32
    R32 = mybir.dt.float32r
    B16 = mybir.dt.bfloat16
    AT = mybir.ActivationFunctionType
    ALU = mybir.AluOpType
    AX = mybir.AxisListType

    def mmr(out, lhsT, rhs, start, stop):
        nc.tensor.matmul(out, lhsT, rhs, start=start, stop=stop)

    # ---------------- DRAM I/O ----------------
    d = {}
    def din(name, shape, dt_=None):
        d[name] = nc.dram_tensor(name, list(shape), dt_ or F32, kind="ExternalInput")
        return d[name]

    din("xin", (128, BPC, KD, C), B16)
    din("memb", (128, KD, DM), B16)
    din("wpos", (C, DM))
    din("ident", (128, 128), R32)
    din("identb", (128, 128), B16)
    din("vw1t", (L, 128, KD, H), B16); din("vw2t", (L, 128, KH, DM), B16)
    din("aw1t", (L, 128, KD, H), B16); din("aw2t", (L, 128, KH, DM), B16)
    din("m1", (L, 128, KD, DM), B16); din("m2", (L, 128, KD, DM), B16)
    din("vb1", (L, 128, KH)); din("ab1", (L, 128, KH))
    din("vb2", (L, DM)); din("ab2", (L, DM))
    din("c1", (L, DM)); din("c2", (L, DM))
    din("vsb", (L, 1)); din("asb", (L, 1))
    din("vgc", (L, C)); din("vbc", (L, C)); din("vgl", (L, DM)); din("vbl", (L, DM))
    din("agc", (L, C)); din("abc", (L, C)); din("agl", (L, DM)); din("abl", (L, DM))
    out_d = nc.dram_tensor("out", [BPC, C, DM], F32, kind="ExternalOutput")

    def bc_ap(src, parts=128):
        return bass.AP(tensor=src.tensor, offset=src.offset,
                       ap=[[0, parts]] + [list(x) for x in src.ap])

    def col_ap(src):
        return bass.AP(tensor=src.tensor, offset=src.offset,
                       ap=[list(src.ap[0]), [0, 1]])

    with tile.TileContext(nc) as tc:
        import contextlib
        ctx = contextlib.ExitStack()
        with ctx:
            wp = ctx.enter_context(tc.tile_pool(name="wp", bufs=2))
            ap_ = ctx.enter_context(tc.tile_pool(name="ap", bufs=1))
            bcp = ctx.enter_context(tc.tile_pool(name="bcp", bufs=8))
            sp = ctx.enter_context(tc.tile_pool(name="sp", bufs=8))
            cp = ctx.enter_context(tc.tile_pool(name="cp", bufs=1))
            pbig = ctx.enter_context(tc.tile_pool(name="pbig", bufs=4, space="PSUM"))
            ph = ctx.enter_context(tc.tile_pool(name="ph", bufs=2, space="PSUM"))
            pt = ctx.enter_context(tc.tile_pool(name="pt", bufs=2, space="PSUM"))

            # ---------------- constants ----------------
            xin_sb = cp.tile([128, BPC, KD, C], B16)
            for c in range(BPC):
                nc.sync.dma_start(out=xin_sb[:, c], in_=d["xin"].ap()[:, c])
            memb_sb = cp.tile([128, KD, DM], B16)
            nc.sync.dma_start(out=memb_sb, in_=d["memb"].ap())
            wpos_sb = cp.tile([128, DM], F32)
            nc.sync.dma_start(out=wpos_sb, in_=d["wpos"].ap())
            ident = cp.tile([128, 128], R32)
            nc.sync.dma_start(out=ident, in_=d["ident"].ap())
            identb = cp.tile([128, 128], B16)
            nc.sync.dma_start(out=identb, in_=d["identb"].ap())
            epsc = cp.tile([128, 1], F32)
            nc.vector.memset(epsc, EPS)
            magic = cp.tile([128, 1], mybir.dt.int32)
            nc.vector.memset(magic, 0x5f3759df)
            c15 = cp.tile([128, 1], F32)
            nc.vector.memset(c15, 1.5)

            # ---------------- embed:  x[c] = xin[c].T @ memb + wpos ----------------
            x_t = ap_.tile([128, BPC, DM], R32, tag="x", bufs=2)
            for c in range(BPC):
                x_ps = pbig.tile([128, DM], F32, tag="big")
                for k in range(KD):
                    mmr(x_ps, xin_sb[:, c, k, :], memb_sb[:, k, :],
                        start=(k == 0), stop=(k == KD - 1))
                nc.vector.tensor_add(x_t[:, c, :], x_ps, wpos_sb)

            inv_sqc = float(1.0 / np.sqrt(DM * np.sqrt(DM)))
            phase = os.environ.get("KPHASE", "full")
            srow_of = {}

            def ve(c):
                return nc.vector if c == 0 else nc.gpsimd

            def psum_copy(c, out, in_):
                # Pool cannot access PSUM: c0 -> DVE, c1 -> ACT Identity
                if c == 0:
                    nc.vector.tensor_copy(out, in_)
                else:
                    nc.scalar.activation(out, in_, AT.Identity)

            # ---------------- layers ----------------
            def load_weights(l):
                w = {}
                w["vw1t"] = wp.tile([128, KD, H], B16, tag="vw1t", name=f"vw1t{l}")
                nc.sync.dma_start(out=w["vw1t"], in_=d["vw1t"][l])
                w["vw2t"] = wp.tile([128, KH, DM], B16, tag="vw2t", name=f"vw2t{l}")
                nc.sync.dma_start(out=w["vw2t"], in_=d["vw2t"][l])
                w["m1"] = wp.tile([128, KD, DM], B16, tag="m1", name=f"m1_{l}")
                nc.sync.dma_start(out=w["m1"], in_=d["m1"][l])
                w["m2"] = wp.tile([128, KD, DM], B16, tag="m2", name=f"m2_{l}")
                nc.sync.dma_start(out=w["m2"], in_=d["m2"][l])
                w["aw1t"] = wp.tile([128, KD, H], B16, tag="aw1t", name=f"aw1t{l}")
                nc.sync.dma_start(out=w["aw1t"], in_=d["aw1t"][l])
                w["aw2t"] = wp.tile([128, KH, DM], B16, tag="aw2t", name=f"aw2t{l}")
                nc.sync.dma_start(out=w["aw2t"], in_=d["aw2t"][l])
                w["vb1"] = sp.tile([128, KH], F32, tag="vb1", bufs=2, name=f"vb1_{l}")
                nc.sync.dma_start(out=w["vb1"], in_=d["vb1"][l])
                w["ab1"] = sp.tile([128, KH], F32, tag="ab1", bufs=2, name=f"ab1_{l}")
                nc.sync.dma_start(out=w["ab1"], in_=d["ab1"][l])
                return w

            wts = load_weights(0) if phase != "emb" else None
            for l in range(L if phase == "full" else 1):
                if phase == "emb":
                    break
                vw1t, vw2t, m1, m2, aw1t, aw2t, vb1, ab1 = (
                    wts["vw1t"], wts["vw2t"], wts["m1"], wts["m2"],
                    wts["aw1t"], wts["aw2t"], wts["vb1"], wts["ab1"])

                vgc = sp.tile([128, 1], F32, tag="vgc")
                nc.gpsimd.dma_start(out=vgc, in_=col_ap(d["vgc"][l]))
                agc = sp.tile([128, 1], F32, tag="agc")
                nc.gpsimd.dma_start(out=agc, in_=col_ap(d["agc"][l]))
                vbc = sp.tile([128, 1], F32, tag="vbc")
                nc.gpsimd.dma_start(out=vbc, in_=col_ap(d["vbc"][l]))
                abc = sp.tile([128, 1], F32, tag="abc")
                nc.gpsimd.dma_start(out=abc, in_=col_ap(d["abc"][l]))

                def bcast(name):
                    t = bcp.tile([128, DM], F32, tag="bc", name=f"{name}_bc{l}")
                    eng = nc.gpsimd if l == 0 else nc.sync
                    eng.dma_start(out=t, in_=bc_ap(d[name][l]))
                    return t
                vglb = bcast("vgl"); vblb = bcast("vbl")
                aglb = bcast("agl"); ablb = bcast("abl")
                vb2b = bcast("vb2"); ab2b = bcast("ab2")
                c1b = bcast("c1"); c2b = bcast("c2")

                gcI = sp.tile([128, 128], R32, tag="gcI", bufs=2)
                nc.vector.tensor_scalar_mul(gcI, ident, vgc)
                vsb = sp.tile([128, 1], F32, tag="vsb")
                nc.gpsimd.dma_start(out=vsb, in_=bc_ap(d["vsb"][l]))
                asb = sp.tile([128, 1], F32, tag="asb")
                nc.gpsimd.dma_start(out=asb, in_=bc_ap(d["asb"][l]))

                # ============ VarCor block ============
                cT = ap_.tile([128, BPC, 128], R32, tag="cT")
                for c in range(BPC):
                    if c in srow_of:
                        srow = srow_of[c]
                    else:
                        srow = sp.tile([128, 1], F32, tag="srow", bufs=4)
                        nc.vector.tensor_reduce(srow, x_t[:, c, :], AX.X, ALU.add)
                    s2 = sp.tile([128, 1], R32, tag="s2", bufs=4)
                    nc.scalar.mul(s2, srow, inv_sqc)
                    sT_ps = pbig.tile([1, 128], R32, tag="big", name=f"sTps{l}_{c}")
                    nc.tensor.transpose(sT_ps, s2, ident)
                    sT = sp.tile([1, 128], R32, tag="sT", bufs=4)
                    nc.scalar.activation(sT, sT_ps, AT.Identity)
                    corr_ps = pbig.tile([128, 128], F32, tag="big", name=f"corrps{l}_{c}")
                    mmr(corr_ps, sT, sT, start=True, stop=True)
                    corrE = ap_.tile([128, 128], F32, tag="corrE", bufs=2)
                    rsum = sp.tile([128, 1], F32, tag="rsum", bufs=4)
                    nc.scalar.activation(corrE, corr_ps, AT.Exp, accum_out=rsum)
                    rinv = sp.tile([128, 1], F32, tag="rinv", bufs=4)
                    nc.vector.reciprocal(rinv, rsum)
                    corrBN = ap_.tile([128, 128], R32, tag="corrBN", bufs=2)
                    nc.vector.tensor_scalar(corrBN, corrE, rinv, vgc, ALU.mult, ALU.mult)
                    # cT = corrBN^T + gcI^T  (gcI diagonal: transpose == itself)
                    cT_ps = pbig.tile([128, 128], R32, tag="big", name=f"cTps{l}_{c}")
                    nc.tensor.matmul(cT_ps, corrBN, ident, is_transpose=True,
                                     start=True, stop=False)
                    nc.tensor.matmul(cT_ps, gcI, ident, is_transpose=True,
                                     start=False, stop=True)
                    psum_copy(c, cT[:, c, :], cT_ps)

                # prewarm Gelu table; input aliases c1's corrE so the
                # scheduler cannot hoist this above the last Exp use
                pw_g = sp.tile([128, 1], F32, tag="pw", bufs=4, name=f"pwg{l}")
                nc.scalar.activation(pw_g, corrE[:, 0:1], AT.Gelu)

                # r2 rows-major via corr matmul; feature-major via transposes
                r2r = ap_.tile([128, BPC, DM], B16, tag="r2r")
                r2T = ap_.tile([128, KD, 2 * 128], B16, tag="r2T")
                for c in range(BPC):
                    rr_ps = pbig.tile([128, DM], F32, tag="big", name=f"rrps{l}_{c}")
                    mmr(rr_ps, cT[:, c, :], x_t[:, c, :], start=True, stop=True)
                    nc.scalar.activation(r2r[:, c, :], rr_ps, AT.Identity, bias=vbc)
                    rt_ps = pt.tile([128, KD, 128], B16, tag="t", name=f"rtps{l}_{c}")
                    for m in range(KD):
                        nc.tensor.matmul(rt_ps[:, m, :],
                                         r2r[:, c, m * 128:(m + 1) * 128], identb,
                                         is_transpose=True, start=True, stop=True)
                    psum_copy(c, r2T[:, :, c * 128:(c + 1) * 128], rt_ps)

                if phase == "corr":
                    x_t = r2r
                    break
                x_t = _ffn_ln(nc, tile, mybir, bass, tc, ap_, sp, bcp, ph, pbig, pt,
                              r2T, r2r, vw1t, vb1, vw2t, vb2b, vglb, vblb, l, "v",
                              epsc, vsb, srow_of, ident, magic, c15, ve,
                              dup=True)
                if phase == "vc0":
                    break
                if l + 1 < L and phase == "full":
                    wts = load_weights(l + 1)

                # ============ Auto-attention block ============
                # x_t here is x_dup [128, BPC, 2*DM]; x view is first half
                x_dup = x_t
                def xv(c):
                    return x_dup[:, c, 0:DM]

                # xT feature-major via batched transposes
                xT = ap_.tile([128, KD, 2 * 128], B16, tag="xT")
                for c in range(BPC):
                    xt_ps = pt.tile([128, KD, 128], B16, tag="t", name=f"xtps{l}_{c}")
                    for m in range(KD):
                        nc.tensor.matmul(xt_ps[:, m, :],
                                         xv(c)[:, m * 128:(m + 1) * 128], identb,
                                         is_transpose=True, start=True, stop=True)
                    psum_copy(c, xT[:, :, c * 128:(c + 1) * 128], xt_ps)

                # u = x @ M1 + c1   (rows-major out; c1 via ones-fold)
                u_t = ap_.tile([128, BPC, DM], F32, tag="u")
                for c in range(BPC):
                    u_ps = pbig.tile([128, DM], F32, tag="big", name=f"ups{l}_{c}")
                    for k in range(KD):
                        mmr(u_ps, xT[:, k, c * 128:(c + 1) * 128], m1[:, k, :],
                            start=(k == 0), stop=(k == KD - 1))
                    nc.vector.tensor_add(u_t[:, c, :], u_ps, c1b)

                if phase == "u":
                    x_t = u_t
                    break

                # scores S[r,i] = <u, roll_i(x)> * DM^-0.5 ; softmax over i
                scl = float(DM ** -0.5)
                Se_of = {}
                sinv_of = {}
                for c in range(BPC):
                    trash = ap_.tile([128, DM], F32, tag="trash", bufs=1,
                                     name=f"tr{l}_{c}")
                    Sa = sp.tile([128, NS], F32, tag="Sa", bufs=2)
                    for i in range(NS):
                        nc.vector.scalar_tensor_tensor(
                            out=trash, in0=u_t[:, c, :], scalar=scl,
                            in1=x_dup[:, c, P * i:P * i + DM],
                            op0=ALU.mult, op1=ALU.mult,
                            accum_out=Sa[:, i:i + 1])
                    Se = sp.tile([128, NS], F32, tag="Se", bufs=2,
                                 name=f"Se{l}_{c}")
                    ssum = sp.tile([128, 1], F32, tag="ssum", bufs=4)
                    nc.scalar.activation(Se, Sa, AT.Exp, accum_out=ssum)
                    sinv = sp.tile([128, 1], F32, tag="sinv", bufs=4,
                                   name=f"sinv{l}_{c}")
                    nc.vector.reciprocal(sinv, ssum)
                    Se_of[c] = Se
                    sinv_of[c] = sinv

                # prewarm Gelu for the a-FFN; dep-anchored on c1's Se
                pw_g2 = sp.tile([128, 1], F32, tag="pw", bufs=4, name=f"pwg2{l}")
                nc.scalar.activation(pw_g2, Se_of[BPC - 1][:, 0:1], AT.Gelu)

                if phase == "sc":
                    x_t = ap_.tile([128, BPC, DM], F32, tag="scdump")
                    nc.vector.memset(x_t, 0.0)
                    for c in range(BPC):
                        nc.vector.tensor_scalar(x_t[:, c, 0:NS], Se_of[c],
                                                sinv_of[c], None, ALU.mult)
                    break

                # vm = sum_i p_i roll_i(x): diag(p_i) bf16, x_dup slices wide
                vm_t = ap_.tile([128, BPC, DM], B16, tag="vm")
                for c in range(BPC):
                    dgs = ap_.tile([128, NS, 128], B16, tag="dg", bufs=2,
                                   name=f"dgs{l}_{c}")
                    for i in range(NS):
                        if c == 0:
                            nc.vector.tensor_scalar_mul(dgs[:, i, :], identb,
                                                        Se_of[c][:, i:i + 1])
                        else:
                            nc.scalar.activation(dgs[:, i, :], identb, AT.Identity,
                                                 scale=Se_of[c][:, i:i + 1])
                    vm_ps = pbig.tile([128, DM], F32, tag="big", name=f"vmps{l}_{c}")
                    for i in range(NS):
                        mmr(vm_ps, dgs[:, i, :], x_dup[:, c, P * i:P * i + DM],
                            start=(i == 0), stop=(i == NS - 1))
                    # normalize by 1/sum(exp) fused into the PSUM evacuation
                    nc.vector.tensor_scalar(vm_t[:, c, :], vm_ps, sinv_of[c],
                                            None, ALU.mult)

                if phase == "vm":
                    x_t = vm_t
                    break

                # vmT feature-major
                vmT = ap_.tile([128, KD, 2 * 128], B16, tag="vmT")
                for c in range(BPC):
                    vt_ps = pt.tile([128, KD, 128], B16, tag="t", name=f"vtps{l}_{c}")
                    for m in range(KD):
                        nc.tensor.matmul(vt_ps[:, m, :],
                                         vm_t[:, c, m * 128:(m + 1) * 128], identb,
                                         is_transpose=True, start=True, stop=True)
                    psum_copy(c, vmT[:, :, c * 128:(c + 1) * 128], vt_ps)

                # o = vm @ M2 + c2 + x (residual via ident-fold); r1 = BN(o)
                # xc2 = x + c2 precomputed off the critical path (Pool)
                xc2 = ap_.tile([128, BPC, DM], F32, tag="xc2")
                for c in range(BPC):
                    nc.gpsimd.tensor_add(xc2[:, c, :], xv(c), c2b)
                r1r = ap_.tile([128, BPC, DM], B16, tag="r1r")
                for c in range(BPC):
                    o_ps = pbig.tile([128, DM], F32, tag="big", name=f"ops{l}_{c}")
                    for k in range(KD):
                        mmr(o_ps, vmT[:, k, c * 128:(c + 1) * 128], m2[:, k, :],
                            start=(k == 0), stop=(k == KD - 1))
                    t1 = ap_.tile([128, DM], F32, tag="t1", bufs=2, name=f"t1{l}_{c}")
                    nc.vector.tensor_add(t1, o_ps, xc2[:, c, :])
                    nc.scalar.activation(r1r[:, c, :], t1, AT.Identity,
                                         bias=abc, scale=agc)

                if phase == "attn":
                    x_t = r1r
                    break

                # r1T feature-major
                r1T = ap_.tile([128, KD, 2 * 128], B16, tag="r1T")
                for c in range(BPC):
                    r1_ps = pt.tile([128, KD, 128], B16, tag="t", name=f"r1ps{l}_{c}")
                    for m in range(KD):
                        nc.tensor.matmul(r1_ps[:, m, :],
                                         r1r[:, c, m * 128:(m + 1) * 128], identb,
                                         is_transpose=True, start=True, stop=True)
                    psum_copy(c, r1T[:, :, c * 128:(c + 1) * 128], r1_ps)

                x_t = _ffn_ln(nc, tile, mybir, bass, tc, ap_, sp, bcp, ph, pbig, pt,
                              r1T, r1r, aw1t, ab1, aw2t, ab2b, aglb, ablb, l, "a",
                              epsc, asb, srow_of, ident, magic, c15, ve,
                              dup=False)

            # ---------------- store ----------------
            for c in range(BPC):
                nc.sync.dma_start(out=out_d.ap()[c],
                                  in_=x_t[:, c, 0:DM].bitcast(F32))


def _ffn_ln(nc, tile, mybir, bass, tc, ap_, sp, bcp, ph, pbig, pt,
            rT, rrows, w1t, b1c, w2t, b2b, glb, blb, l, pfx, epsc,
            sumb, srow_of, ident, magic, c15, ve, dup):
    """h = gelu(r @ W1.T + b1); y = h @ W2.T + b2; x = LN(y + r) * g + b.

    b1/b2/residual folded into the PSUM accumulation. LN stats via bn_stats
    on the PSUM z; rstd without the ACT Sqrt table (KRSTD=pow) or with it
    (KRSTD=sqrt). If dup, output x is [128, BPC, 2*DM] (two copies side by
    side) for the shift-window consumers."""
    F32 = mybir.dt.float32
    R32 = mybir.dt.float32r
    B16 = mybir.dt.bfloat16
    AT = mybir.ActivationFunctionType
    ALU = mybir.AluOpType

    # FFN1: hT hidden-major; split by c so each batch chain can run its
    # FFN while the other is still in attention; b1 via per-mh gelu bias
    hT = ap_.tile([128, KH, 2 * 128], B16, tag="hT", bufs=2, name=f"hT{pfx}{l}")
    for c in range(BPC):
        for mh2 in range(KH // 2):
            h_ps = ph.tile([128, 4, 128], F32, tag="h",
                           name=f"hps{pfx}{l}_{c}_{mh2}")
            for half in range(2):
                mh = mh2 * 2 + half
                for k in range(KD):
                    nc.tensor.matmul(h_ps[:, 2 * c + half, :],
                                     w1t[:, k, mh * 128:(mh + 1) * 128],
                                     rT[:, k, c * 128:(c + 1) * 128],
                                     start=(k == 0), stop=(k == KD - 1))
                nc.scalar.activation(hT[:, mh, c * 128:(c + 1) * 128],
                                     h_ps[:, 2 * c + half, :], AT.Gelu,
                                     bias=b1c[:, mh:mh + 1])

    xw = 2 * DM if dup else DM
    x_new = ap_.tile([128, BPC, xw], B16 if dup else R32,
                     tag="xd" if dup else "x", bufs=2, name=f"x{pfx}{l}")
    # rb = r + b2 precomputed off the critical path (Pool)
    rb = ap_.tile([128, BPC, DM], F32, tag="rb", bufs=1, name=f"rb{pfx}{l}")
    for c in range(BPC):
        nc.gpsimd.tensor_add(rb[:, c, :], rrows[:, c, :], b2b)
    for c in range(BPC):
        y_ps = pbig.tile([128, DM], F32, tag="big", name=f"yps{pfx}{l}_{c}")
        for k in range(KH):
            nc.tensor.matmul(y_ps, hT[:, k, c * 128:(c + 1) * 128],
                             w2t[:, k, :], start=(k == 0), stop=(k == KH - 1))
        z = ap_.tile([128, DM], F32, tag="z", bufs=2, name=f"z{pfx}{l}_{c}")
        nc.vector.tensor_add(z, y_ps, rb[:, c, :])
        st6 = sp.tile([128, 6], F32, tag="st6", bufs=4)
        nc.vector.bn_stats(out=st6, in_=z)
        mv = sp.tile([128, 2], F32, tag="mv", bufs=4)
        nc.vector.bn_aggr(out=mv, in_=st6)
        vef = sp.tile([128, 1], F32, tag="vef", bufs=4)
        nc.vector.tensor_scalar(vef, mv[:, 1:2], EPS, None, ALU.add)
        veh = sp.tile([128, 1], F32, tag="veh", bufs=4)
        nc.vector.tensor_scalar(veh, vef, 0.5, None, ALU.mult)
        yi = sp.tile([128, 1], mybir.dt.int32, tag="yi", bufs=4)
        nc.vector.tensor_scalar(yi, vef.bitcast(mybir.dt.int32), 1, None,
                                ALU.logical_shift_right)
        y0 = sp.tile([128, 1], mybir.dt.int32, tag="y0", bufs=4)
        nc.vector.tensor_sub(y0, magic, yi)
        y0f = y0.bitcast(F32)
        q0 = sp.tile([128, 1], F32, tag="q0", bufs=4)
        nc.vector.tensor_mul(q0, y0f, y0f)
        w0 = sp.tile([128, 1], F32, tag="w0", bufs=4)
        nc.vector.scalar_tensor_tensor(out=w0, in0=q0, scalar=veh, in1=c15,
                                       op0=ALU.mult, op1=ALU.subtract)
        y1 = sp.tile([128, 1], F32, tag="y1", bufs=4)
        nc.vector.tensor_mul(y1, y0f, w0)
        q1 = sp.tile([128, 1], F32, tag="q1", bufs=4)
        nc.vector.tensor_mul(q1, y1, y1)
        w1 = sp.tile([128, 1], F32, tag="w1", bufs=4)
        nc.vector.scalar_tensor_tensor(out=w1, in0=q1, scalar=veh, in1=c15,
                                       op0=ALU.mult, op1=ALU.subtract)
        rstd = sp.tile([128, 1], F32, tag="rstd", bufs=4)
        nc.vector.tensor_mul(rstd, y1, w1)
        xn = ap_.tile([128, DM], F32, tag="xn", bufs=2, name=f"xn{pfx}{l}_{c}")
        nc.vector.tensor_scalar(xn, z, mv[:, 0:1], rstd, ALU.subtract, ALU.mult)
        if pfx == "a" and l < L - 1:
            trash2 = ap_.tile([128, DM], F32, tag="trash2", bufs=1,
                              name=f"tr2{pfx}{l}_{c}")
            sraw = sp.tile([128, 1], F32, tag="sraw", bufs=4)
            nc.vector.scalar_tensor_tensor(
                out=trash2, in0=xn, scalar=1.0, in1=glb,
                op0=ALU.mult, op1=ALU.mult, accum_out=sraw)
            srow = sp.tile([128, 1], F32, tag="srow", bufs=4, name=f"srow{pfx}{l}_{c}")
            nc.scalar.activation(srow, sraw, AT.Identity, bias=sumb)
            srow_of[c] = srow
        # affine x = xn*g + b  (c0 on DVE, c1 on Pool)
        ve(c).tensor_mul(x_new[:, c, 0:DM], xn, glb)
        ve(c).tensor_add(x_new[:, c, 0:DM], x_new[:, c, 0:DM], blb)
        if dup:
            nc.vector.tensor_copy(x_new[:, c, DM:2 * DM], x_new[:, c, 0:DM])
    # prewarm Exp for the next softmax; dep-anchored on the last gelu tile.
    # Skipped after the final FFN (nothing needs Exp again).
    if not (pfx == "a" and l == L - 1):
        pw_e = sp.tile([128, 1], F32, tag="pw", bufs=4, name=f"pwe{pfx}{l}")
        nc.scalar.activation(pw_e, hT[:, KH - 1, 255:256], AT.Exp)
    return x_new


# ======================================================================
# host side
# ======================================================================

_COMPILED = {}


def _compile():
    if "nc" in _COMPILED:
        return _COMPILED["nc"]
    import concourse.bass as bass
    import concourse.bacc as bacc
    import concourse.tile as tile
    from concourse import mybir
    nc = bacc.Bacc("TRN2", target_bir_lowering=False, debug=False, num_devices=NC_)
    _build(nc, tile, mybir, bass)
    nc.compile()
    _COMPILED["nc"] = nc
    return nc


def _host_prep(inputs):
    import ml_dtypes
    bf16 = ml_dtypes.bfloat16
    f = lambda k: np.asarray(inputs[k], np.float32)
    ld_w = f("ld_w").reshape(KS).astype(np.float64)
    S = np.zeros((T, T), np.float64)
    idx = np.clip(np.arange(T)[:, None] + np.arange(KS)[None, :] - KS // 2, 0, T - 1)
    for k in range(KS):
        np.add.at(S, (np.arange(T), idx[:, k]), ld_w[k])
    Rm = np.eye(T) - S
    emb_W = f("emb_W").astype(np.float64)
    memb = (Rm.T @ emb_W.T).astype(np.float32)              # (T, DM)
    wpos = (f("W_pos") + f("emb_b")[None, :]
            - float(f("ld_b")[0]) * emb_W.sum(1).astype(np.float32)[None, :])

    def shuf(a):
        kn, n = a.shape
        return a.reshape(kn // 128, 128, n).transpose(1, 0, 2)

    g = {"memb": np.ascontiguousarray(shuf(memb)).astype(bf16),
         "wpos": np.ascontiguousarray(wpos.astype(np.float32)),
         "ident": np.eye(128, dtype=np.float32),
         "identb": np.eye(128).astype(bf16)}

    s1 = np.float32(1.0 / np.sqrt(1.0 + EPS))
    def stack(fn, dt=np.float32):
        return np.ascontiguousarray(np.stack([fn(l) for l in range(L)]).astype(dt))

    g["vw1t"] = stack(lambda l: shuf(f("vc_W1")[l].T), bf16)
    g["vw2t"] = stack(lambda l: shuf(f("vc_W2")[l].T), bf16)
    g["aw1t"] = stack(lambda l: shuf(f("aa_W1")[l].T), bf16)
    g["aw2t"] = stack(lambda l: shuf(f("aa_W2")[l].T), bf16)
    g["m1"] = stack(lambda l: shuf(f("aa_Wq")[l].astype(np.float64).T
                                   @ f("aa_Wk")[l].astype(np.float64)), bf16)
    g["m2"] = stack(lambda l: shuf((f("aa_Wo")[l].astype(np.float64)
                                    @ f("aa_Wv")[l].astype(np.float64)).T), bf16)
    g["vb1"] = stack(lambda l: f("vc_b1")[l].reshape(KH, 128).T)
    g["ab1"] = stack(lambda l: f("aa_b1")[l].reshape(KH, 128).T)
    g["vb2"] = stack(lambda l: f("vc_b2")[l])
    g["ab2"] = stack(lambda l: f("aa_b2")[l])
    g["c1"] = stack(lambda l: f("aa_bq")[l].astype(np.float64)
                              @ f("aa_Wk")[l].astype(np.float64))
    g["c2"] = stack(lambda l: f("aa_bv")[l].astype(np.float64)
                              @ f("aa_Wo")[l].astype(np.float64).T
                              + f("aa_bo")[l].astype(np.float64))
    g["vsb"] = stack(lambda l: f("vc_ln_b")[l].sum(keepdims=True))
    g["asb"] = stack(lambda l: f("aa_ln_b")[l].sum(keepdims=True))
    g["vgc"] = stack(lambda l: f("vc_bn_g")[l] * s1)
    g["vbc"] = stack(lambda l: f("vc_bn_b")[l])
    g["vgl"] = stack(lambda l: f("vc_ln_g")[l])
    g["vbl"] = stack(lambda l: f("vc_ln_b")[l])
    g["agc"] = stack(lambda l: f("aa_bn_g")[l] * s1)
    g["abc"] = stack(lambda l: f("aa_bn_b")[l])
    g["agl"] = stack(lambda l: f("aa_ln_g")[l])
    g["abl"] = stack(lambda l: f("aa_ln_b")[l])
    return g


def kernel(**inputs):
    from concourse.bass_utils import run_bass_kernel_spmd
    nc = _compile()
    g = _host_prep(inputs)
    inp = np.asarray(inputs["inp"], np.float32)
    in_maps = []
    for core in range(NC_):
        m = dict(g)
        sl = inp[core * BPC:(core + 1) * BPC]          # (BPC, T, C)
        m["xin"] = np.ascontiguousarray(
            sl.reshape(BPC, KD, 128, C).transpose(2, 0, 1, 3)).astype(g["memb"].dtype)
        in_maps.append(m)
    res = run_bass_kernel_spmd(nc, in_maps, core_ids=list(range(NC_)))
    if res.exec_time_ns is not None:
        kernel.last_exec_time_ns = res.exec_time_ns
    out = np.concatenate([res.results[k]["out"] for k in range(NC_)], axis=0)
    return out


kernel.last_exec_time_ns = None
